# revision 54
# baseline (speedup 1.0000x reference)
"""KoLeo loss kernel for 8 Trainium2 NeuronCores.

Reference semantics:
    v = latents.squeeze()            # [N, D] f32, N=16384, D=64
    dp = v @ v.T ; dp[i,i] = -1      # NxN scores, diagonal excluded
    idx = argmax(dp, axis=1)         # nearest neighbor by dot product
    dist = ||v - v[idx] + 1e-6||_2
    out = mean(relu(-log(dist * N)))

Sharding: rows are block-sharded 2048/core.  Each core gets a copy of v
whose rows are ROTATED by -core*2048, so the self-match diagonal of its
local [2048, 16384] score block always lands at pair-column (row//2) --
the SPMD program is identical on all cores.

Pairwise-max trick: max(a, b) = (a + b + |a-b|) / 2.  The host ships
paired sums w = v[0::2]+v[1::2] and diffs u = v[0::2]-v[1::2] as
fp8-e4m3 scaled by 2^5, and the row operand as bf16 scaled by 2^12
(product scale 2^17, exact); fp8 halves the startup DMA critical path.
The PE computes dp-sums s = rows @ w.T and dp-diffs d = rows @ u.T in
[128,1024] f32 PSUM tiles (waves); A/B row-tiles use disjoint PE array
quadrants (rows 0-63 / 64-127) with MM-level interleave for 2x matmul
concurrency, and the four single-buffered PSUM tags (psSA/psSB/psDA/
psDB) fill all 8 banks, A and B acting as each other's double buffer.
ScalarE takes |d| out of PSUM (its only bulk job -- the DVE can read
just one PSUM stream per op, so the abs cannot fold into the pack), and
a fused custom VectorE op consumes (|d| from SBUF, s from PSUM) at one
output/cycle -- i.e. TWO dp elements per DVE cycle:

    pack = round_16384(s + |d|) + global_pair_idx;  accum = max

The fp32 magic-constant rounding makes the pack exact; the scan carries
the GLOBAL pair index (wave base via the scan init), and the accumulator
is chained across waves through its seed operand, so a row-tile's final
bm value directly encodes (quantized pair-max, winning pair index) over
all 8192 pairs -- no per-chunk argmax recovery pass is needed; a second
one-shot custom op (KOLEO_UNPACK_IDX) turns bm into the int32 gather
index in a single DVE instruction.

The self-pair is excluded by accumulating -2^30 into the SUM stream at
the self position (one extra K=64 quadrant-local matmul per row-tile,
wave 0 only, so the A and B diag MMs overlap on disjoint PE quadrants);
the partner (the excluded pair's other member) is reinstated as an
extra candidate in the tail from host-precomputed partner dot/norm
values.

Startup DMAs: the Scalar engine issues none (each dma_start costs
~0.6-1.2us of hidden descriptor-gen on the issuing engine, and
Scalar's first ABS gates the first pack); SP carries the wave-0
critical set, gpsimd (cheap SWDGE, idle until the first gather) the
tight early waves.

Tail: one indirect gather per finished row-tile fetches the winning
pair's two member vectors plus their precomputed (norm - 2*eps*sum)
values; exact dots pick the member, a host-precomputed partner
candidate is merged, and dist^2 = hr + g - 2*dot, then
relu(-0.5*ln - ln N).  The combine pipeline lags the pack stream by
1-2 s-iterations (stage-1 member products on GpSimd at w==1, stage-2
at w==3 with comparisons on the DVE and the add/sub chain on GpSimd),
koleo for tiles 0-11 ships before the drain, and only tiles 12-15
run after the last scan (entirely on the then-idle DVE).
Host: mean of the 8x2048 per-row values.
"""

import math

import ml_dtypes
import numpy as np

N = 16384
D = 64
NCORES = 8
ROWS = N // NCORES  # 2048 rows per core
P = 128  # partitions
NT = ROWS // P  # 16 row-tiles per core
NPAIR = N // 2  # 8192 pair columns
WCH = 1024  # pair columns per scan wave (2 PSUM banks)
NW = NPAIR // WCH  # 8 waves per row-tile
VREC = 144  # gather record width (f32): 576B, 64B-aligned (130 used)
MM_N = 512  # matmul free dim (1 PSUM bank)
EPS = 1.0e-6

SCALE_R = 4096.0  # 2^12 pre-scale baked into vrows on the host
SCALE_W = 32.0  # 2^5 pre-scale baked into fp8 wt/ut (product 2^17)
BIG = 2.0**30  # diagonal suppression on the (scaled) SUM stream
PACK_MAGIC = 3.0 * 2.0**22 * 16384.0  # rounds to multiples of 16384
UNPACK_MAGIC = 12582912.0  # 3 * 2^22: rounds to integers
NEGINF = -3.0e38

_OP_NAME = "KOLEO_PACK_GIDX"
_IDX_OP_NAME = "KOLEO_UNPACK_IDX"
_built = {}


def _register_idx_op():
    """Register the one-shot index-unpack op (idempotent).

    pfi = Src0 + ((Src0*C0 + C1) - C1)*C2 with C0=1/16384, C1=3*2^22,
    C2=-16384: the inner magic-add rounds bm/16384 to the quantized-max
    part exactly (the index fraction is < 0.5), so the output is the
    GLOBAL winning pair index, written straight to the int32 gather
    operand.  Replaces the old 2x ScalarE copy + DVE add + cast chain
    (and its Scalar->DVE hop) with a single DVE instruction.
    """
    from concourse import dve_ops
    from concourse.dve_spec import AluOp, C0, C1, C2, Spec, Src0, lower
    from concourse.dve_uop import DveOpSpec

    if _IDX_OP_NAME in dve_ops._SUB_OPCODE_FOR_NAME:
        return next(op for op in dve_ops.OPS if op.name == _IDX_OP_NAME)

    def _reference(in0, s0, s1, imm2):
        bm = in0.astype(np.float32)
        t = (bm * np.float32(s0) + np.float32(s1)) - np.float32(s1)
        return (bm + t * np.float32(imm2)).astype(np.float32)

    body = Src0 + (((Src0 * C0) + C1) - C1) * C2
    spec = Spec(
        body=body,
        reference=lambda in0, in1, s0, s1, imm2: _reference(in0, s0, s1, imm2),
    )

    row = max(dve_ops._SUB_OPCODE_FOR_NAME.values()) + 1
    shas = {}
    for ver in ("v3", "v4"):
        uops = lower(spec, ver=ver)
        shas[ver] = DveOpSpec(
            name=_IDX_OP_NAME, opcode=row, uops=uops, rd1_en=False
        ).sha(ver)

    op = dve_ops.DveOp(_IDX_OP_NAME, spec, subdim=False, uops_sha=shas)
    dve_ops.OPS.append(op)
    dve_ops._SUB_OPCODE_FOR_NAME[_IDX_OP_NAME] = row
    dve_ops.CUSTOM_DVE_SPECS[_IDX_OP_NAME] = spec
    return op


def _register_pack_op():
    """Register the fused pair-max/global-argmax custom DVE op (idempotent).

    body  = round_16384(Src0 + Src1) + scan(+1, init=C1)
    accum = max, seeded from C0 (chains across waves via bm)
    Src0 = 2^17-scaled |dp-diffs| (SBUF), Src1 = 2^17-scaled dp-sums
    (PSUM); the DVE can read only ONE stream from PSUM (NCC_IBVF027), so
    the abs runs as a ScalarE pass (its only bulk job, 140us < the DVE's
    158us of packs, so it hides).
    C1 = wave_base - 1 so the scan value is the GLOBAL pair index.
    C2 (imm2) = PACK_MAGIC.
    """
    from concourse import dve_ops
    from concourse.dve_spec import (
        AluOp, C0, C1, C2, One, Spec, Src0, Src1, lower, scan,
    )
    from concourse.dve_uop import DveOpSpec

    if _OP_NAME in dve_ops._SUB_OPCODE_FOR_NAME:
        return next(op for op in dve_ops.OPS if op.name == _OP_NAME)

    def _reference(in0, in1, s0, s1, imm2):
        p = in0.shape[0]
        s = in0.astype(np.float32).reshape(p, -1)
        a = in1.astype(np.float32).reshape(p, s.shape[1])
        z = (s + a) + np.float32(imm2) - np.float32(imm2)
        col = (np.float32(s1) + 1.0 + np.arange(s.shape[1], dtype=np.float32))[
            None, :
        ]
        body = (z + col).astype(np.float32)
        seed = np.asarray(s0, dtype=np.float32)
        seed = (
            seed.reshape(p, -1)[:, :1]
            if seed.ndim
            else np.full((p, 1), seed, dtype=np.float32)
        )
        acc = np.maximum(body.max(axis=-1, keepdims=True), seed)
        return body, acc

    body = ((Src0 + Src1) + C2 - C2) + scan(AluOp.ADD, One, init=C1)
    spec = Spec(
        body=body, accum=AluOp.MAX, accum_init=C0, reference=_reference
    )

    row = max(dve_ops._SUB_OPCODE_FOR_NAME.values()) + 1
    shas = {}
    for ver in ("v3", "v4"):
        uops = lower(spec, ver=ver)
        shas[ver] = DveOpSpec(
            name=_OP_NAME, opcode=row, uops=uops, rd1_en=True
        ).sha(ver)

    op = dve_ops.DveOp(_OP_NAME, spec, subdim=False, uops_sha=shas)
    dve_ops.OPS.append(op)
    dve_ops._SUB_OPCODE_FOR_NAME[_OP_NAME] = row
    dve_ops.CUSTOM_DVE_SPECS[_OP_NAME] = spec
    return op


def _build_nc():
    """Build + compile the per-core Bass program (same NEFF on all cores)."""
    if "nc" in _built:
        return _built["nc"]

    import concourse.bass as bass
    import concourse.mybir as mybir
    import concourse.tile as tile
    from concourse import bacc

    pack_op = _register_pack_op()
    idx_op = _register_idx_op()

    f32 = mybir.dt.float32
    bf16 = mybir.dt.bfloat16
    i32 = mybir.dt.int32
    Alu = mybir.AluOpType
    Act = mybir.ActivationFunctionType

    nc = bacc.Bacc(None, target_bir_lowering=False)

    f8 = mybir.dt.float8e4
    # wt/ut/vrows_t are duplicated into partitions 64-127 for row-packing
    wt_d = nc.declare_dram_parameter("wt", [P, NPAIR], f8, isOutput=False)
    ut_d = nc.declare_dram_parameter("ut", [P, NPAIR], f8, isOutput=False)
    vrows_t = nc.declare_dram_parameter("vrows_t", [P, ROWS], bf16, isOutput=False)
    # rows duplicated along D so one op handles both gathered pair members
    vr2_d = nc.declare_dram_parameter("vr2", [P, NT, 2 * D], f32, isOutput=False)
    # pair record: [v[2p] | v[2p+1] | g_a | g_b | pad pad] (132 f32 = 528B)
    vpair_d = nc.declare_dram_parameter("vpair", [NPAIR, VREC], f32, isOutput=False)
    hr_d = nc.declare_dram_parameter("hr", [P, NT], f32, isOutput=False)
    pd_d = nc.declare_dram_parameter("pd", [P, NT], f32, isOutput=False)
    pg_d = nc.declare_dram_parameter("pg", [P, NT], f32, isOutput=False)
    out_d = nc.declare_dram_parameter("out", [P, NT], f32, isOutput=True)
    if DEBUG:
        dbg_bm = nc.declare_dram_parameter("dbg_bm", [P, NT], f32, isOutput=True)
        dbg_pf = nc.declare_dram_parameter("dbg_pf", [P, NT], f32, isOutput=True)
        dbg_dq = nc.declare_dram_parameter("dbg_dq", [P, NT, 2], f32, isOutput=True)
        dbg_d2 = nc.declare_dram_parameter("dbg_d2", [P, NT], f32, isOutput=True)
        dbg_g = nc.declare_dram_parameter("dbg_g", [P, NT, 4], f32, isOutput=True)

    # diag suppression as K=64 quadrant-local matmuls: out[r, t*64+r//2]
    # += -BIG via lhsT[r//2, r] = -BIG against a 64x64 identity, duplicated
    # into partitions 64-127 so the A/B diag MMs overlap on disjoint PE
    # quadrants instead of a full-128-row matmul blocking both
    nh_np = np.zeros((P // 2, P), dtype=np.float32)
    nh_np[np.arange(P) // 2, np.arange(P)] = -BIG
    neg_half = nc.inline_tensor(
        np.concatenate([nh_np, nh_np], axis=0).astype(ml_dtypes.bfloat16),
        "neg_half",
    )
    eye64_np = np.concatenate(
        [np.eye(P // 2, dtype=np.float32)] * 2, axis=0
    )
    eye64 = nc.inline_tensor(eye64_np.astype(ml_dtypes.bfloat16), "eye64")
    neginf_np = np.full((P, NT), NEGINF, dtype=np.float32)
    neginf_c = nc.inline_tensor(neginf_np, "neginf")
    mlnn_np = np.full((P, 1), -math.log(float(N)), dtype=np.float32)
    mlnn_c = nc.inline_tensor(mlnn_np, "mlnn")

    with tile.TileContext(nc) as tc:
        with (
            tc.tile_pool(name="consts", bufs=1) as consts,
            tc.tile_pool(name="psum", bufs=1, space="PSUM") as psum_pool,
            tc.tile_pool(name="work", bufs=1) as work,
            tc.tile_pool(name="small", bufs=1) as small,
        ):
            # ---- startup DMAs.  The Scalar (Activation) engine issues NONE:
            # each dma_start costs ~0.6us of hidden HWDGE generation on the
            # issuing engine, and Scalar's first ABS gates the first pack.
            # SP has no compute duties, so it takes the wave-0 critical set
            # and most of the rest; gpsimd (SWDGE on Q7) takes the odd waves
            # and the high-descriptor-count vr2 -- it is idle until the
            # first gather at ~30us.
            vrt_sb = consts.tile([P, ROWS], bf16)
            wt_sb = consts.tile([P, NPAIR], f8)
            ut_sb = consts.tile([P, NPAIR], f8)
            neginf_sb = consts.tile([P, NT], f32)
            nh_sb = consts.tile([P, P], bf16)
            eye64_sb = consts.tile([P, P // 2], bf16)
            vr2_sb = consts.tile([P, NT, 2 * D], f32)
            hr_sb = consts.tile([P, NT], f32)
            pd_sb = consts.tile([P, NT], f32)
            pg_sb = consts.tile([P, NT], f32)
            mlnn_sb = consts.tile([P, 1], f32)

            def wsl(w):
                return slice(w * WCH, (w + 1) * WCH)

            # SP pays ~1.2us per dma_start (seq + HWDGE) while gpsimd's SWDGE
            # gen is ~0.65us and its queue is otherwise idle until the first
            # gather (~32us), so gpsimd carries waves 1-3,5 (tight deadlines)
            # and SP the wave-0 critical set + the late waves
            # wave-0 in halves so the first D-matmul starts on ut0-h0 while
            # h1 is still in flight; the tiny diag consts ride gpsimd's
            # cheaper queue ahead of wave 1
            h0 = slice(0, MM_N)
            h1 = slice(MM_N, WCH)
            nc.sync.dma_start(vrt_sb[:, 0:256], vrows_t[:, 0:256])
            nc.sync.dma_start(ut_sb[:, h0], ut_d[:, h0])
            nc.sync.dma_start(ut_sb[:, h1], ut_d[:, h1])
            nc.sync.dma_start(wt_sb[:, h0], wt_d[:, h0])
            nc.sync.dma_start(wt_sb[:, h1], wt_d[:, h1])
            nc.gpsimd.dma_start(nh_sb[:], neg_half[:])
            nc.gpsimd.dma_start(eye64_sb[:], eye64[:])
            nc.gpsimd.dma_start(wt_sb[:, wsl(1)], wt_d[:, wsl(1)])
            nc.gpsimd.dma_start(ut_sb[:, wsl(1)], ut_d[:, wsl(1)])
            nc.gpsimd.dma_start(neginf_sb[:], neginf_c[:])
            nc.gpsimd.dma_start(wt_sb[:, wsl(2)], wt_d[:, wsl(2)])
            nc.gpsimd.dma_start(ut_sb[:, wsl(2)], ut_d[:, wsl(2)])
            nc.gpsimd.dma_start(wt_sb[:, wsl(3)], wt_d[:, wsl(3)])
            nc.gpsimd.dma_start(ut_sb[:, wsl(3)], ut_d[:, wsl(3)])
            nc.sync.dma_start(vrt_sb[:, 256:1024], vrows_t[:, 256:1024])
            nc.gpsimd.dma_start(wt_sb[:, wsl(5)], wt_d[:, wsl(5)])
            nc.gpsimd.dma_start(ut_sb[:, wsl(5)], ut_d[:, wsl(5)])
            nc.sync.dma_start(wt_sb[:, wsl(4)], wt_d[:, wsl(4)])
            nc.sync.dma_start(ut_sb[:, wsl(4)], ut_d[:, wsl(4)])
            nc.sync.dma_start(vrt_sb[:, 1024:ROWS], vrows_t[:, 1024:ROWS])
            nc.gpsimd.dma_start(vr2_sb[:], vr2_d[:])
            nc.sync.dma_start(wt_sb[:, wsl(6)], wt_d[:, wsl(6)])
            nc.sync.dma_start(ut_sb[:, wsl(6)], ut_d[:, wsl(6)])
            nc.sync.dma_start(wt_sb[:, wsl(7)], wt_d[:, wsl(7)])
            nc.sync.dma_start(ut_sb[:, wsl(7)], ut_d[:, wsl(7)])
            nc.sync.dma_start(hr_sb[:], hr_d[:])
            nc.sync.dma_start(pd_sb[:], pd_d[:])
            nc.gpsimd.dma_start(pg_sb[:], pg_d[:])
            nc.sync.dma_start(mlnn_sb[:], mlnn_c[:])

            bm = small.tile([P, NT], f32)  # packed per-tile running maxima
            pfi = small.tile([P, NT], i32)  # winning pair index (int)
            gat = small.tile([P, NT, VREC], f32)  # gathered pair records
            prj = small.tile([P, NT, 2, D], f32)  # member products
            dq = small.tile([P, NT, 2], f32)  # member dots
            sel = small.tile([P, NT], f32)  # 1.0 if member b has larger dot
            dotw = small.tile([P, NT], f32)
            gdf = small.tile([P, NT], f32)
            gm = small.tile([P, NT], f32)
            pgd = small.tile([P, NT], f32)
            pgm = small.tile([P, NT], f32)
            gw = small.tile([P, NT], f32)
            c2 = small.tile([P, NT], f32)
            dotf = small.tile([P, NT], f32)
            gf = small.tile([P, NT], f32)
            s2a = small.tile([P, NT], f32)
            s2b = small.tile([P, NT], f32)
            dist2 = small.tile([P, NT], f32)

            def wave_pair(s, w):
                tA, tB = 2 * s, 2 * s + 1
                lhsA = vrt_sb[0:64, tA * P : (tA + 1) * P]
                lhsB = vrt_sb[64:128, tB * P : (tB + 1) * P]
                if True:
                    # A uses PE rows 0-63, B rows 64-127: adjacent MMs
                    # alternate quadrants so pairs run concurrently.  The 4
                    # single-buffered tag-tiles fill all 8 PSUM banks; A and
                    # B act as each other's double buffer in the pipeline.
                    psSA = psum_pool.tile([P, WCH], f32, tag="psSA", bufs=1)
                    psSB = psum_pool.tile([P, WCH], f32, tag="psSB", bufs=1)
                    psDA = psum_pool.tile([P, WCH], f32, tag="psDA", bufs=1)
                    psDB = psum_pool.tile([P, WCH], f32, tag="psDB", bufs=1)
                    for h in range(WCH // MM_N):
                        csl = slice(w * WCH + h * MM_N, w * WCH + (h + 1) * MM_N)
                        osl = slice(h * MM_N, (h + 1) * MM_N)
                        nc.tensor.matmul(
                            psDA[:, osl], lhsA, ut_sb[0:64, csl],
                            start=True, stop=True,
                        )
                        nc.tensor.matmul(
                            psDB[:, osl], lhsB, ut_sb[64:128, csl],
                            start=True, stop=True,
                        )
                    absA = work.tile([P, WCH], f32, tag="absD", bufs=4)
                    nc.scalar.activation(absA[:], psDA[:], Act.Abs)
                    for h in range(WCH // MM_N):
                        csl = slice(w * WCH + h * MM_N, w * WCH + (h + 1) * MM_N)
                        osl = slice(h * MM_N, (h + 1) * MM_N)
                        # the self-pair block (cols [t*64, t*64+64)) is
                        # always inside wave 0; keep that MM's group open
                        dhA = w == 0 and h == (tA * 64) // MM_N
                        dhB = w == 0 and h == (tB * 64) // MM_N
                        nc.tensor.matmul(
                            psSA[:, osl], lhsA, wt_sb[0:64, csl],
                            start=True, stop=not dhA,
                        )
                        nc.tensor.matmul(
                            psSB[:, osl], lhsB, wt_sb[64:128, csl],
                            start=True, stop=not dhB,
                        )
                    if w == 0:
                        offA, offB = tA * 64, tB * 64
                        nc.tensor.matmul(
                            psSA[:, offA : offA + 64],
                            nh_sb[0:64, :], eye64_sb[0:64, :],
                            start=False, stop=True,
                        )
                        nc.tensor.matmul(
                            psSB[:, offB : offB + 64],
                            nh_sb[64:128, :], eye64_sb[64:128, :],
                            start=False, stop=True,
                        )
                    absB = work.tile([P, WCH], f32, tag="absD", bufs=4)
                    nc.scalar.activation(absB[:], psDB[:], Act.Abs)
                    for t, psS, absD in ((tA, psSA, absA), (tB, psSB, absB)):
                        junk = work.tile([P, WCH], f32, tag="junk", bufs=2)
                        seed = (
                            neginf_sb[:, t : t + 1]
                            if w == 0
                            else bm[:, t : t + 1]
                        )
                        nc.vector._custom_dve(
                            pack_op, out=junk[:], in0=absD[:], in1=psS[:],
                            s0=seed, s1=float(w * WCH - 1), imm2=PACK_MAGIC,
                            accum_out=bm[:, t : t + 1],
                        )
                        if w == NW - 1:
                            # tile finished: recover its winning pair index
                            # with ONE fused DVE op (writes the int32 gather
                            # operand directly) and launch the gather.  It
                            # must stay on the DVE: bm is WAR-tracked at tile
                            # granularity, so another engine reading it would
                            # stall the next pack whenever that engine is
                            # mid-gather.  For the final tile pair the
                            # scheduler otherwise floats both idx ops past
                            # the last pack, starting gather 14 ~1.4us late
                            # on the drain-critical path -- pin them early
                            ts1 = slice(t, t + 1)
                            import contextlib
                            prio = (
                                tc.high_priority()
                                if s == NT // 2 - 1
                                else contextlib.nullcontext()
                            )
                            with prio:
                                nc.vector._custom_dve(
                                    idx_op, out=pfi[:, ts1], in0=bm[:, ts1],
                                    s0=1.0 / 16384.0, s1=UNPACK_MAGIC,
                                    imm2=-16384.0,
                                )
                                nc.gpsimd.indirect_dma_start(
                                    out=gat[:, t, :], out_offset=None,
                                    in_=vpair_d[:],
                                    in_offset=bass.IndirectOffsetOnAxis(
                                        ap=pfi[:, ts1], axis=0
                                    ),
                                )

            def tail_mult(g4):
                # stage 1 (GpSimd only): member products from gathered pairs
                nc.gpsimd.tensor_tensor(
                    out=prj[:, g4, :, :],
                    in0=vr2_sb[:, g4, :],
                    in1=gat[:, g4, 0 : 2 * D],
                    op=Alu.mult,
                )

            def tail_rest(g4):
                # stage 2.  DVE ops depend only on prj (produced one
                # s-iteration ago) and each other, so they never wait on
                # GpSimd; the GpSimd chain is pure add/sub/mult (the only
                # Pool-supported TT ALU ops) and absorbs the cross-engine
                # waits itself.
                nc.vector.tensor_reduce(
                    dq[:, g4, :], prj[:, g4, :, :],
                    axis=mybir.AxisListType.X, op=Alu.add,
                )
                # member with larger dot; its g via predicated copy
                nc.vector.tensor_tensor(
                    out=sel[:, g4], in0=dq[:, g4, 1], in1=dq[:, g4, 0],
                    op=Alu.is_gt,
                )
                nc.vector.tensor_tensor(
                    out=dotw[:, g4], in0=dq[:, g4, 0], in1=dq[:, g4, 1],
                    op=Alu.max,
                )
                # partner candidate wins if its (host-exact) dot is larger
                nc.vector.tensor_tensor(
                    out=c2[:, g4], in0=pd_sb[:, g4], in1=dotw[:, g4],
                    op=Alu.is_gt,
                )
                nc.vector.tensor_tensor(
                    out=dotf[:, g4], in0=dotw[:, g4], in1=pd_sb[:, g4],
                    op=Alu.max,
                )
                nc.gpsimd.tensor_tensor(
                    out=gdf[:, g4], in0=gat[:, g4, 2 * D + 1],
                    in1=gat[:, g4, 2 * D], op=Alu.subtract,
                )
                nc.gpsimd.tensor_tensor(
                    out=gm[:, g4], in0=gdf[:, g4], in1=sel[:, g4], op=Alu.mult
                )
                nc.gpsimd.tensor_tensor(
                    out=gw[:, g4], in0=gat[:, g4, 2 * D], in1=gm[:, g4],
                    op=Alu.add,
                )
                nc.gpsimd.tensor_tensor(
                    out=pgd[:, g4], in0=pg_sb[:, g4], in1=gw[:, g4],
                    op=Alu.subtract,
                )
                nc.gpsimd.tensor_tensor(
                    out=pgm[:, g4], in0=pgd[:, g4], in1=c2[:, g4], op=Alu.mult
                )
                nc.gpsimd.tensor_tensor(
                    out=gf[:, g4], in0=gw[:, g4], in1=pgm[:, g4], op=Alu.add
                )
                # dist^2 = hr + g - 2*dot, as (hr - dot) - dot + g so the
                # whole chain stays on Pool-legal add/sub ops
                nc.gpsimd.tensor_tensor(
                    out=s2a[:, g4], in0=hr_sb[:, g4], in1=dotf[:, g4],
                    op=Alu.subtract,
                )
                nc.gpsimd.tensor_tensor(
                    out=s2b[:, g4], in0=s2a[:, g4], in1=dotf[:, g4],
                    op=Alu.subtract,
                )
                nc.gpsimd.tensor_tensor(
                    out=dist2[:, g4], in0=s2b[:, g4], in1=gf[:, g4],
                    op=Alu.add,
                )

            def tail_rest_dve(g4, with_mult=True):
                # drain variant: the pack stream is over, the DVE is idle --
                # run the whole chain (incl. stage-1 mult) on it with zero
                # cross-engine hops (GpSimd is busy with the last gathers)
                if with_mult:
                    nc.vector.tensor_tensor(
                        out=prj[:, g4, :, :],
                        in0=vr2_sb[:, g4, :],
                        in1=gat[:, g4, 0 : 2 * D],
                        op=Alu.mult,
                    )
                nc.vector.tensor_reduce(
                    dq[:, g4, :], prj[:, g4, :, :],
                    axis=mybir.AxisListType.X, op=Alu.add,
                )
                nc.vector.tensor_tensor(
                    out=sel[:, g4], in0=dq[:, g4, 1], in1=dq[:, g4, 0],
                    op=Alu.is_gt,
                )
                nc.vector.tensor_tensor(
                    out=dotw[:, g4], in0=dq[:, g4, 0], in1=dq[:, g4, 1],
                    op=Alu.max,
                )
                nc.vector.tensor_tensor(
                    out=c2[:, g4], in0=pd_sb[:, g4], in1=dotw[:, g4],
                    op=Alu.is_gt,
                )
                nc.vector.tensor_tensor(
                    out=dotf[:, g4], in0=dotw[:, g4], in1=pd_sb[:, g4],
                    op=Alu.max,
                )
                nc.vector.tensor_tensor(
                    out=gdf[:, g4], in0=gat[:, g4, 2 * D + 1],
                    in1=gat[:, g4, 2 * D], op=Alu.subtract,
                )
                nc.vector.tensor_tensor(
                    out=gm[:, g4], in0=gdf[:, g4], in1=sel[:, g4], op=Alu.mult
                )
                nc.vector.tensor_tensor(
                    out=gw[:, g4], in0=gat[:, g4, 2 * D], in1=gm[:, g4],
                    op=Alu.add,
                )
                nc.vector.tensor_tensor(
                    out=pgd[:, g4], in0=pg_sb[:, g4], in1=gw[:, g4],
                    op=Alu.subtract,
                )
                nc.vector.tensor_tensor(
                    out=pgm[:, g4], in0=pgd[:, g4], in1=c2[:, g4], op=Alu.mult
                )
                nc.vector.tensor_tensor(
                    out=gf[:, g4], in0=gw[:, g4], in1=pgm[:, g4], op=Alu.add
                )
                nc.vector.scalar_tensor_tensor(
                    out=s2a[:, g4], in0=dotf[:, g4], scalar=-2.0,
                    in1=gf[:, g4], op0=Alu.mult, op1=Alu.add,
                )
                nc.vector.tensor_tensor(
                    out=dist2[:, g4], in0=s2a[:, g4], in1=hr_sb[:, g4],
                    op=Alu.add,
                )

            lns = small.tile([P, NT], f32)
            kol = small.tile([P, NT], f32)

            for s in range(NT // 2):
                tA, tB = 2 * s, 2 * s + 1
                for w in range(NW):
                    wave_pair(s, w)
                    # combines lag the pack stream: stage 1 for the 2 tiles
                    # finished at s-1, stage 2 one s-iteration later.  Both
                    # are emitted mid-iteration (after waves 1 and 3) so
                    # they sit in the GpSimd queue ahead of this iteration's
                    # own unpack/gather burst -- that keeps prj(12,13) ready
                    # before the drain and stage-2's DVE ops stall-free
                    if w == 1 and s >= 1:
                        tail_mult(slice(2 * s - 2, 2 * s))
                    if w == 3 and s >= 2:
                        tail_rest(slice(2 * s - 4, 2 * s - 2))

            # dist2 for tiles 0-11 is final before the drain: ship their
            # koleo values while the last gathers are still in flight
            nc.scalar.activation(lns[:, 0:12], dist2[:, 0:12], Act.Ln)
            nc.scalar.activation(
                kol[:, 0:12], lns[:, 0:12], Act.Relu,
                bias=mlnn_sb[:, 0:1], scale=-0.5,
            )
            nc.sync.dma_start(out_d[:, 0:12], kol[:, 0:12])

            # drain: tiles 12/13 combine while gathers 14/15 finish (their
            # stage-1 mult was emitted at s=7 w1); 14/15 run as ONE 2-wide
            # chain -- the DVE is in-order so two per-tile chains would
            # serialize anyway, and both gathers land within ~1us of each
            # other, so one chain over both tiles is ~1us shorter
            tail_rest_dve(slice(12, 14), with_mult=False)
            tail_rest_dve(slice(14, NT))
            nc.scalar.activation(lns[:, 12:NT], dist2[:, 12:NT], Act.Ln)
            # koleo = relu(-0.5*ln(dist^2) - ln(N))
            nc.scalar.activation(
                kol[:, 12:NT], lns[:, 12:NT], Act.Relu,
                bias=mlnn_sb[:, 0:1], scale=-0.5,
            )
            # gpsimd SWDGE is ~2us cheaper than SP's HWDGE for this last
            # latency-exposed 2KB store
            nc.gpsimd.dma_start(out_d[:, 12:NT], kol[:, 12:NT])
            if DEBUG:
                nc.sync.dma_start(dbg_bm[:], bm[:])
                nc.sync.dma_start(dbg_pf[:], pfi[:])
                nc.sync.dma_start(dbg_dq[:], dq[:])
                nc.sync.dma_start(dbg_d2[:], dist2[:])
                nc.sync.dma_start(dbg_g[:, :, 0], gat[:, :, 128])
                nc.sync.dma_start(dbg_g[:, :, 1], gat[:, :, 129])
                nc.sync.dma_start(dbg_g[:, :, 2], gat[:, :, 0])
                nc.sync.dma_start(dbg_g[:, :, 3], gat[:, :, D])


    nc.compile()
    _built["nc"] = nc
    return nc


def _prep_in_maps(v: np.ndarray) -> list[dict]:
    bf = ml_dtypes.bfloat16
    f8 = ml_dtypes.float8_e4m3
    in_maps = []
    jj = np.arange(ROWS)
    for c in range(NCORES):
        vr = np.roll(v, -c * ROWS, axis=0)
        w = (vr[0::2] + vr[1::2]) * np.float32(SCALE_W)  # [NPAIR, D]
        u = (vr[0::2] - vr[1::2]) * np.float32(SCALE_W)
        rows = vr[:ROWS]
        wt = np.ascontiguousarray(w.T).astype(f8)
        ut = np.ascontiguousarray(u.T).astype(f8)
        rt = np.ascontiguousarray(rows.T * np.float32(SCALE_R)).astype(bf)
        rsb = rows.reshape(NT, P, D).transpose(1, 0, 2)

        n2 = np.sum(vr.astype(np.float64) * vr, axis=1).astype(np.float32)
        Sv = np.sum(vr.astype(np.float64), axis=1).astype(np.float32)
        gvec = n2 - np.float32(2.0 * EPS) * Sv  # per-row: ||x||^2 - 2 eps sum(x)

        vpair = np.zeros((NPAIR, VREC), dtype=np.float32)
        vpair[:, 0:D] = vr[0::2]
        vpair[:, D : 2 * D] = vr[1::2]
        vpair[:, 2 * D] = gvec[0::2]
        vpair[:, 2 * D + 1] = gvec[1::2]

        hr = (
            n2[:ROWS]
            + np.float32(2.0 * EPS) * Sv[:ROWS]
            + np.float32(D * EPS * EPS)
        )
        pidx = jj ^ 1
        pdot = np.sum(
            rows.astype(np.float64) * vr[pidx], axis=1
        ).astype(np.float32)
        pgv = gvec[pidx]

        in_maps.append(
            {
                "wt": np.concatenate([wt, wt], axis=0),
                "ut": np.concatenate([ut, ut], axis=0),
                "vrows_t": np.concatenate([rt, rt], axis=0),
                "vr2": np.ascontiguousarray(np.concatenate([rsb, rsb], axis=2)),
                "vpair": vpair,
                "hr": np.ascontiguousarray(hr.reshape(NT, P).T),
                "pd": np.ascontiguousarray(pdot.reshape(NT, P).T),
                "pg": np.ascontiguousarray(pgv.reshape(NT, P).T),
            }
        )
    return in_maps


# test.py can flip these to profile the run
TRACE = False
DEBUG = False
DEBUG_NO_CLAMP = False
LAST_RESULT = {}


def kernel(latents: np.ndarray) -> np.ndarray:
    from concourse.bass_utils import run_bass_kernel_spmd

    v = np.asarray(latents, dtype=np.float32).reshape(N, D)
    nc = _build_nc()
    in_maps = _prep_in_maps(v)

    kwargs = {}
    if TRACE:
        kwargs = dict(trace=True, stitch_traces=False)
    res = run_bass_kernel_spmd(nc, in_maps, core_ids=list(range(NCORES)), **kwargs)
    LAST_RESULT["res"] = res

    vals = np.concatenate([r["out"].reshape(-1) for r in res.results])
    return np.array(np.mean(vals), dtype=np.float32)


# revision 55
# speedup vs baseline: 1.0003x; 1.0003x over previous
"""KoLeo loss kernel for 8 Trainium2 NeuronCores.

Reference semantics:
    v = latents.squeeze()            # [N, D] f32, N=16384, D=64
    dp = v @ v.T ; dp[i,i] = -1      # NxN scores, diagonal excluded
    idx = argmax(dp, axis=1)         # nearest neighbor by dot product
    dist = ||v - v[idx] + 1e-6||_2
    out = mean(relu(-log(dist * N)))

Sharding: rows are block-sharded 2048/core.  Each core gets a copy of v
whose rows are ROTATED by -core*2048, so the self-match diagonal of its
local [2048, 16384] score block always lands at pair-column (row//2) --
the SPMD program is identical on all cores.

Pairwise-max trick: max(a, b) = (a + b + |a-b|) / 2.  The host ships
paired sums w = v[0::2]+v[1::2] and diffs u = v[0::2]-v[1::2] as
fp8-e4m3 scaled by 2^5, and the row operand as bf16 scaled by 2^12
(product scale 2^17, exact); fp8 halves the startup DMA critical path.
The PE computes dp-sums s = rows @ w.T and dp-diffs d = rows @ u.T in
[128,1024] f32 PSUM tiles (waves); A/B row-tiles use disjoint PE array
quadrants (rows 0-63 / 64-127) with MM-level interleave for 2x matmul
concurrency, and the four single-buffered PSUM tags (psSA/psSB/psDA/
psDB) fill all 8 banks, A and B acting as each other's double buffer.
ScalarE takes |d| out of PSUM (its only bulk job -- the DVE can read
just one PSUM stream per op, so the abs cannot fold into the pack), and
a fused custom VectorE op consumes (|d| from SBUF, s from PSUM) at one
output/cycle -- i.e. TWO dp elements per DVE cycle:

    pack = round_16384(s + |d|) + global_pair_idx;  accum = max

The fp32 magic-constant rounding makes the pack exact; the scan carries
the GLOBAL pair index (wave base via the scan init), and the accumulator
is chained across waves through its seed operand, so a row-tile's final
bm value directly encodes (quantized pair-max, winning pair index) over
all 8192 pairs -- no per-chunk argmax recovery pass is needed; a second
one-shot custom op (KOLEO_UNPACK_IDX) turns bm into the int32 gather
index in a single DVE instruction.

The self-pair is excluded by accumulating -2^30 into the SUM stream at
the self position (one extra K=64 quadrant-local matmul per row-tile,
wave 0 only, so the A and B diag MMs overlap on disjoint PE quadrants);
the partner (the excluded pair's other member) is reinstated as an
extra candidate in the tail from host-precomputed partner dot/norm
values.

Startup DMAs: the Scalar engine issues none (each dma_start costs
~0.6-1.2us of hidden descriptor-gen on the issuing engine, and
Scalar's first ABS gates the first pack); SP carries the wave-0
critical set, gpsimd (cheap SWDGE, idle until the first gather) the
tight early waves.

Tail: one indirect gather per finished row-tile fetches the winning
pair's two member vectors plus their precomputed (norm - 2*eps*sum)
values; exact dots pick the member, a host-precomputed partner
candidate is merged, and dist^2 = hr + g - 2*dot, then
relu(-0.5*ln - ln N).  The combine pipeline lags the pack stream by
1-2 s-iterations (stage-1 member products on GpSimd at w==1, stage-2
at w==3 with comparisons on the DVE and the add/sub chain on GpSimd),
koleo for tiles 0-11 ships before the drain, and only tiles 12-15
run after the last scan (entirely on the then-idle DVE).
Host: mean of the 8x2048 per-row values.
"""

import math

import ml_dtypes
import numpy as np

N = 16384
D = 64
NCORES = 8
ROWS = N // NCORES  # 2048 rows per core
P = 128  # partitions
NT = ROWS // P  # 16 row-tiles per core
NPAIR = N // 2  # 8192 pair columns
WCH = 1024  # pair columns per scan wave (2 PSUM banks)
NW = NPAIR // WCH  # 8 waves per row-tile
VREC = 144  # gather record width (f32): 576B, 64B-aligned (130 used)
MM_N = 512  # matmul free dim (1 PSUM bank)
EPS = 1.0e-6

SCALE_R = 4096.0  # 2^12 pre-scale baked into vrows on the host
SCALE_W = 32.0  # 2^5 pre-scale baked into fp8 wt/ut (product 2^17)
BIG = 2.0**30  # diagonal suppression on the (scaled) SUM stream
PACK_MAGIC = 3.0 * 2.0**22 * 16384.0  # rounds to multiples of 16384
UNPACK_MAGIC = 12582912.0  # 3 * 2^22: rounds to integers
NEGINF = -3.0e38

_OP_NAME = "KOLEO_PACK_GIDX"
_IDX_OP_NAME = "KOLEO_UNPACK_IDX"
_built = {}


def _register_idx_op():
    """Register the one-shot index-unpack op (idempotent).

    pfi = Src0 + ((Src0*C0 + C1) - C1)*C2 with C0=1/16384, C1=3*2^22,
    C2=-16384: the inner magic-add rounds bm/16384 to the quantized-max
    part exactly (the index fraction is < 0.5), so the output is the
    GLOBAL winning pair index, written straight to the int32 gather
    operand.  Replaces the old 2x ScalarE copy + DVE add + cast chain
    (and its Scalar->DVE hop) with a single DVE instruction.
    """
    from concourse import dve_ops
    from concourse.dve_spec import AluOp, C0, C1, C2, Spec, Src0, lower
    from concourse.dve_uop import DveOpSpec

    if _IDX_OP_NAME in dve_ops._SUB_OPCODE_FOR_NAME:
        return next(op for op in dve_ops.OPS if op.name == _IDX_OP_NAME)

    def _reference(in0, s0, s1, imm2):
        bm = in0.astype(np.float32)
        t = (bm * np.float32(s0) + np.float32(s1)) - np.float32(s1)
        return (bm + t * np.float32(imm2)).astype(np.float32)

    body = Src0 + (((Src0 * C0) + C1) - C1) * C2
    spec = Spec(
        body=body,
        reference=lambda in0, in1, s0, s1, imm2: _reference(in0, s0, s1, imm2),
    )

    row = max(dve_ops._SUB_OPCODE_FOR_NAME.values()) + 1
    shas = {}
    for ver in ("v3", "v4"):
        uops = lower(spec, ver=ver)
        shas[ver] = DveOpSpec(
            name=_IDX_OP_NAME, opcode=row, uops=uops, rd1_en=False
        ).sha(ver)

    op = dve_ops.DveOp(_IDX_OP_NAME, spec, subdim=False, uops_sha=shas)
    dve_ops.OPS.append(op)
    dve_ops._SUB_OPCODE_FOR_NAME[_IDX_OP_NAME] = row
    dve_ops.CUSTOM_DVE_SPECS[_IDX_OP_NAME] = spec
    return op


def _register_pack_op():
    """Register the fused pair-max/global-argmax custom DVE op (idempotent).

    body  = round_16384(Src0 + Src1) + scan(+1, init=C1)
    accum = max, seeded from C0 (chains across waves via bm)
    Src0 = 2^17-scaled |dp-diffs| (SBUF), Src1 = 2^17-scaled dp-sums
    (PSUM); the DVE can read only ONE stream from PSUM (NCC_IBVF027), so
    the abs runs as a ScalarE pass (its only bulk job, 140us < the DVE's
    158us of packs, so it hides).
    C1 = wave_base - 1 so the scan value is the GLOBAL pair index.
    C2 (imm2) = PACK_MAGIC.
    """
    from concourse import dve_ops
    from concourse.dve_spec import (
        AluOp, C0, C1, C2, One, Spec, Src0, Src1, lower, scan,
    )
    from concourse.dve_uop import DveOpSpec

    if _OP_NAME in dve_ops._SUB_OPCODE_FOR_NAME:
        return next(op for op in dve_ops.OPS if op.name == _OP_NAME)

    def _reference(in0, in1, s0, s1, imm2):
        p = in0.shape[0]
        s = in0.astype(np.float32).reshape(p, -1)
        a = in1.astype(np.float32).reshape(p, s.shape[1])
        z = (s + a) + np.float32(imm2) - np.float32(imm2)
        col = (np.float32(s1) + 1.0 + np.arange(s.shape[1], dtype=np.float32))[
            None, :
        ]
        body = (z + col).astype(np.float32)
        seed = np.asarray(s0, dtype=np.float32)
        seed = (
            seed.reshape(p, -1)[:, :1]
            if seed.ndim
            else np.full((p, 1), seed, dtype=np.float32)
        )
        acc = np.maximum(body.max(axis=-1, keepdims=True), seed)
        return body, acc

    body = ((Src0 + Src1) + C2 - C2) + scan(AluOp.ADD, One, init=C1)
    spec = Spec(
        body=body, accum=AluOp.MAX, accum_init=C0, reference=_reference
    )

    row = max(dve_ops._SUB_OPCODE_FOR_NAME.values()) + 1
    shas = {}
    for ver in ("v3", "v4"):
        uops = lower(spec, ver=ver)
        shas[ver] = DveOpSpec(
            name=_OP_NAME, opcode=row, uops=uops, rd1_en=True
        ).sha(ver)

    op = dve_ops.DveOp(_OP_NAME, spec, subdim=False, uops_sha=shas)
    dve_ops.OPS.append(op)
    dve_ops._SUB_OPCODE_FOR_NAME[_OP_NAME] = row
    dve_ops.CUSTOM_DVE_SPECS[_OP_NAME] = spec
    return op


def _build_nc():
    """Build + compile the per-core Bass program (same NEFF on all cores)."""
    if "nc" in _built:
        return _built["nc"]

    import concourse.bass as bass
    import concourse.mybir as mybir
    import concourse.tile as tile
    from concourse import bacc

    pack_op = _register_pack_op()
    idx_op = _register_idx_op()

    f32 = mybir.dt.float32
    bf16 = mybir.dt.bfloat16
    i32 = mybir.dt.int32
    Alu = mybir.AluOpType
    Act = mybir.ActivationFunctionType

    nc = bacc.Bacc(None, target_bir_lowering=False)

    f8 = mybir.dt.float8e4
    # wt/ut/vrows_t are duplicated into partitions 64-127 for row-packing
    wt_d = nc.declare_dram_parameter("wt", [P, NPAIR], f8, isOutput=False)
    ut_d = nc.declare_dram_parameter("ut", [P, NPAIR], f8, isOutput=False)
    vrows_t = nc.declare_dram_parameter("vrows_t", [P, ROWS], bf16, isOutput=False)
    # rows duplicated along D so one op handles both gathered pair members
    vr2_d = nc.declare_dram_parameter("vr2", [P, NT, 2 * D], f32, isOutput=False)
    # pair record: [v[2p] | v[2p+1] | g_a | g_b | pad pad] (132 f32 = 528B)
    vpair_d = nc.declare_dram_parameter("vpair", [NPAIR, VREC], f32, isOutput=False)
    hr_d = nc.declare_dram_parameter("hr", [P, NT], f32, isOutput=False)
    pd_d = nc.declare_dram_parameter("pd", [P, NT], f32, isOutput=False)
    pg_d = nc.declare_dram_parameter("pg", [P, NT], f32, isOutput=False)
    out_d = nc.declare_dram_parameter("out", [P, NT], f32, isOutput=True)
    if DEBUG:
        dbg_bm = nc.declare_dram_parameter("dbg_bm", [P, NT], f32, isOutput=True)
        dbg_pf = nc.declare_dram_parameter("dbg_pf", [P, NT], f32, isOutput=True)
        dbg_dq = nc.declare_dram_parameter("dbg_dq", [P, NT, 2], f32, isOutput=True)
        dbg_d2 = nc.declare_dram_parameter("dbg_d2", [P, NT], f32, isOutput=True)
        dbg_g = nc.declare_dram_parameter("dbg_g", [P, NT, 4], f32, isOutput=True)

    # diag suppression as K=64 quadrant-local matmuls: out[r, t*64+r//2]
    # += -BIG via lhsT[r//2, r] = -BIG against a 64x64 identity, duplicated
    # into partitions 64-127 so the A/B diag MMs overlap on disjoint PE
    # quadrants instead of a full-128-row matmul blocking both
    nh_np = np.zeros((P // 2, P), dtype=np.float32)
    nh_np[np.arange(P) // 2, np.arange(P)] = -BIG
    neg_half = nc.inline_tensor(
        np.concatenate([nh_np, nh_np], axis=0).astype(ml_dtypes.bfloat16),
        "neg_half",
    )
    eye64_np = np.concatenate(
        [np.eye(P // 2, dtype=np.float32)] * 2, axis=0
    )
    eye64 = nc.inline_tensor(eye64_np.astype(ml_dtypes.bfloat16), "eye64")
    neginf_np = np.full((P, NT), NEGINF, dtype=np.float32)
    neginf_c = nc.inline_tensor(neginf_np, "neginf")
    mlnn_np = np.full((P, 1), -math.log(float(N)), dtype=np.float32)
    mlnn_c = nc.inline_tensor(mlnn_np, "mlnn")

    with tile.TileContext(nc) as tc:
        with (
            tc.tile_pool(name="consts", bufs=1) as consts,
            tc.tile_pool(name="psum", bufs=1, space="PSUM") as psum_pool,
            tc.tile_pool(name="work", bufs=1) as work,
            tc.tile_pool(name="small", bufs=1) as small,
        ):
            # ---- startup DMAs.  The Scalar (Activation) engine issues NONE:
            # each dma_start costs ~0.6us of hidden HWDGE generation on the
            # issuing engine, and Scalar's first ABS gates the first pack.
            # SP has no compute duties, so it takes the wave-0 critical set
            # and most of the rest; gpsimd (SWDGE on Q7) takes the odd waves
            # and the high-descriptor-count vr2 -- it is idle until the
            # first gather at ~30us.
            vrt_sb = consts.tile([P, ROWS], bf16)
            wt_sb = consts.tile([P, NPAIR], f8)
            ut_sb = consts.tile([P, NPAIR], f8)
            neginf_sb = consts.tile([P, NT], f32)
            nh_sb = consts.tile([P, P], bf16)
            eye64_sb = consts.tile([P, P // 2], bf16)
            vr2_sb = consts.tile([P, NT, 2 * D], f32)
            hr_sb = consts.tile([P, NT], f32)
            pd_sb = consts.tile([P, NT], f32)
            pg_sb = consts.tile([P, NT], f32)
            mlnn_sb = consts.tile([P, 1], f32)

            def wsl(w):
                return slice(w * WCH, (w + 1) * WCH)

            # SP pays ~1.2us per dma_start (seq + HWDGE) while gpsimd's SWDGE
            # gen is ~0.65us and its queue is otherwise idle until the first
            # gather (~32us), so gpsimd carries waves 1-3,5 (tight deadlines)
            # and SP the wave-0 critical set + the late waves
            nc.sync.dma_start(vrt_sb[:, 0:256], vrows_t[:, 0:256])
            nc.sync.dma_start(ut_sb[:, wsl(0)], ut_d[:, wsl(0)])
            nc.sync.dma_start(wt_sb[:, wsl(0)], wt_d[:, wsl(0)])
            nc.sync.dma_start(nh_sb[:], neg_half[:])
            nc.sync.dma_start(eye64_sb[:], eye64[:])
            nc.gpsimd.dma_start(wt_sb[:, wsl(1)], wt_d[:, wsl(1)])
            nc.gpsimd.dma_start(ut_sb[:, wsl(1)], ut_d[:, wsl(1)])
            nc.gpsimd.dma_start(neginf_sb[:], neginf_c[:])
            nc.gpsimd.dma_start(wt_sb[:, wsl(2)], wt_d[:, wsl(2)])
            nc.gpsimd.dma_start(ut_sb[:, wsl(2)], ut_d[:, wsl(2)])
            nc.gpsimd.dma_start(wt_sb[:, wsl(3)], wt_d[:, wsl(3)])
            nc.gpsimd.dma_start(ut_sb[:, wsl(3)], ut_d[:, wsl(3)])
            nc.sync.dma_start(vrt_sb[:, 256:1024], vrows_t[:, 256:1024])
            nc.gpsimd.dma_start(wt_sb[:, wsl(5)], wt_d[:, wsl(5)])
            nc.gpsimd.dma_start(ut_sb[:, wsl(5)], ut_d[:, wsl(5)])
            nc.sync.dma_start(wt_sb[:, wsl(4)], wt_d[:, wsl(4)])
            nc.sync.dma_start(ut_sb[:, wsl(4)], ut_d[:, wsl(4)])
            nc.sync.dma_start(vrt_sb[:, 1024:ROWS], vrows_t[:, 1024:ROWS])
            nc.gpsimd.dma_start(vr2_sb[:], vr2_d[:])
            nc.sync.dma_start(wt_sb[:, wsl(6)], wt_d[:, wsl(6)])
            nc.sync.dma_start(ut_sb[:, wsl(6)], ut_d[:, wsl(6)])
            nc.sync.dma_start(wt_sb[:, wsl(7)], wt_d[:, wsl(7)])
            nc.sync.dma_start(ut_sb[:, wsl(7)], ut_d[:, wsl(7)])
            nc.sync.dma_start(hr_sb[:], hr_d[:])
            nc.sync.dma_start(pd_sb[:], pd_d[:])
            nc.gpsimd.dma_start(pg_sb[:], pg_d[:])
            nc.sync.dma_start(mlnn_sb[:], mlnn_c[:])

            bm = small.tile([P, NT], f32)  # packed per-tile running maxima
            pfi = small.tile([P, NT], i32)  # winning pair index (int)
            gat = small.tile([P, NT, VREC], f32)  # gathered pair records
            prj = small.tile([P, NT, 2, D], f32)  # member products
            dq = small.tile([P, NT, 2], f32)  # member dots
            sel = small.tile([P, NT], f32)  # 1.0 if member b has larger dot
            dotw = small.tile([P, NT], f32)
            gdf = small.tile([P, NT], f32)
            gm = small.tile([P, NT], f32)
            pgd = small.tile([P, NT], f32)
            pgm = small.tile([P, NT], f32)
            gw = small.tile([P, NT], f32)
            c2 = small.tile([P, NT], f32)
            dotf = small.tile([P, NT], f32)
            gf = small.tile([P, NT], f32)
            s2a = small.tile([P, NT], f32)
            s2b = small.tile([P, NT], f32)
            dist2 = small.tile([P, NT], f32)

            def wave_pair(s, w):
                tA, tB = 2 * s, 2 * s + 1
                lhsA = vrt_sb[0:64, tA * P : (tA + 1) * P]
                lhsB = vrt_sb[64:128, tB * P : (tB + 1) * P]
                if True:
                    # A uses PE rows 0-63, B rows 64-127: adjacent MMs
                    # alternate quadrants so pairs run concurrently.  The 4
                    # single-buffered tag-tiles fill all 8 PSUM banks; A and
                    # B act as each other's double buffer in the pipeline.
                    psSA = psum_pool.tile([P, WCH], f32, tag="psSA", bufs=1)
                    psSB = psum_pool.tile([P, WCH], f32, tag="psSB", bufs=1)
                    psDA = psum_pool.tile([P, WCH], f32, tag="psDA", bufs=1)
                    psDB = psum_pool.tile([P, WCH], f32, tag="psDB", bufs=1)
                    for h in range(WCH // MM_N):
                        csl = slice(w * WCH + h * MM_N, w * WCH + (h + 1) * MM_N)
                        osl = slice(h * MM_N, (h + 1) * MM_N)
                        nc.tensor.matmul(
                            psDA[:, osl], lhsA, ut_sb[0:64, csl],
                            start=True, stop=True,
                        )
                        nc.tensor.matmul(
                            psDB[:, osl], lhsB, ut_sb[64:128, csl],
                            start=True, stop=True,
                        )
                    absA = work.tile([P, WCH], f32, tag="absD", bufs=4)
                    nc.scalar.activation(absA[:], psDA[:], Act.Abs)
                    for h in range(WCH // MM_N):
                        csl = slice(w * WCH + h * MM_N, w * WCH + (h + 1) * MM_N)
                        osl = slice(h * MM_N, (h + 1) * MM_N)
                        # the self-pair block (cols [t*64, t*64+64)) is
                        # always inside wave 0; keep that MM's group open
                        dhA = w == 0 and h == (tA * 64) // MM_N
                        dhB = w == 0 and h == (tB * 64) // MM_N
                        nc.tensor.matmul(
                            psSA[:, osl], lhsA, wt_sb[0:64, csl],
                            start=True, stop=not dhA,
                        )
                        nc.tensor.matmul(
                            psSB[:, osl], lhsB, wt_sb[64:128, csl],
                            start=True, stop=not dhB,
                        )
                    if w == 0:
                        offA, offB = tA * 64, tB * 64
                        nc.tensor.matmul(
                            psSA[:, offA : offA + 64],
                            nh_sb[0:64, :], eye64_sb[0:64, :],
                            start=False, stop=True,
                        )
                        nc.tensor.matmul(
                            psSB[:, offB : offB + 64],
                            nh_sb[64:128, :], eye64_sb[64:128, :],
                            start=False, stop=True,
                        )
                    absB = work.tile([P, WCH], f32, tag="absD", bufs=4)
                    nc.scalar.activation(absB[:], psDB[:], Act.Abs)
                    for t, psS, absD in ((tA, psSA, absA), (tB, psSB, absB)):
                        junk = work.tile([P, WCH], f32, tag="junk", bufs=2)
                        seed = (
                            neginf_sb[:, t : t + 1]
                            if w == 0
                            else bm[:, t : t + 1]
                        )
                        nc.vector._custom_dve(
                            pack_op, out=junk[:], in0=absD[:], in1=psS[:],
                            s0=seed, s1=float(w * WCH - 1), imm2=PACK_MAGIC,
                            accum_out=bm[:, t : t + 1],
                        )
                        if w == NW - 1:
                            # tile finished: recover its winning pair index
                            # with ONE fused DVE op (writes the int32 gather
                            # operand directly) and launch the gather.  It
                            # must stay on the DVE: bm is WAR-tracked at tile
                            # granularity, so another engine reading it would
                            # stall the next pack whenever that engine is
                            # mid-gather.  For the final tile pair the
                            # scheduler otherwise floats both idx ops past
                            # the last pack, starting gather 14 ~1.4us late
                            # on the drain-critical path -- pin them early
                            ts1 = slice(t, t + 1)
                            import contextlib
                            prio = (
                                tc.high_priority()
                                if s == NT // 2 - 1
                                else contextlib.nullcontext()
                            )
                            with prio:
                                nc.vector._custom_dve(
                                    idx_op, out=pfi[:, ts1], in0=bm[:, ts1],
                                    s0=1.0 / 16384.0, s1=UNPACK_MAGIC,
                                    imm2=-16384.0,
                                )
                                nc.gpsimd.indirect_dma_start(
                                    out=gat[:, t, :], out_offset=None,
                                    in_=vpair_d[:],
                                    in_offset=bass.IndirectOffsetOnAxis(
                                        ap=pfi[:, ts1], axis=0
                                    ),
                                )

            def tail_mult(g4):
                # stage 1 (GpSimd only): member products from gathered pairs
                nc.gpsimd.tensor_tensor(
                    out=prj[:, g4, :, :],
                    in0=vr2_sb[:, g4, :],
                    in1=gat[:, g4, 0 : 2 * D],
                    op=Alu.mult,
                )

            def tail_rest(g4):
                # stage 2.  DVE ops depend only on prj (produced one
                # s-iteration ago) and each other, so they never wait on
                # GpSimd; the GpSimd chain is pure add/sub/mult (the only
                # Pool-supported TT ALU ops) and absorbs the cross-engine
                # waits itself.
                nc.vector.tensor_reduce(
                    dq[:, g4, :], prj[:, g4, :, :],
                    axis=mybir.AxisListType.X, op=Alu.add,
                )
                # member with larger dot; its g via predicated copy
                nc.vector.tensor_tensor(
                    out=sel[:, g4], in0=dq[:, g4, 1], in1=dq[:, g4, 0],
                    op=Alu.is_gt,
                )
                nc.vector.tensor_tensor(
                    out=dotw[:, g4], in0=dq[:, g4, 0], in1=dq[:, g4, 1],
                    op=Alu.max,
                )
                # partner candidate wins if its (host-exact) dot is larger
                nc.vector.tensor_tensor(
                    out=c2[:, g4], in0=pd_sb[:, g4], in1=dotw[:, g4],
                    op=Alu.is_gt,
                )
                nc.vector.tensor_tensor(
                    out=dotf[:, g4], in0=dotw[:, g4], in1=pd_sb[:, g4],
                    op=Alu.max,
                )
                nc.gpsimd.tensor_tensor(
                    out=gdf[:, g4], in0=gat[:, g4, 2 * D + 1],
                    in1=gat[:, g4, 2 * D], op=Alu.subtract,
                )
                nc.gpsimd.tensor_tensor(
                    out=gm[:, g4], in0=gdf[:, g4], in1=sel[:, g4], op=Alu.mult
                )
                nc.gpsimd.tensor_tensor(
                    out=gw[:, g4], in0=gat[:, g4, 2 * D], in1=gm[:, g4],
                    op=Alu.add,
                )
                nc.gpsimd.tensor_tensor(
                    out=pgd[:, g4], in0=pg_sb[:, g4], in1=gw[:, g4],
                    op=Alu.subtract,
                )
                nc.gpsimd.tensor_tensor(
                    out=pgm[:, g4], in0=pgd[:, g4], in1=c2[:, g4], op=Alu.mult
                )
                nc.gpsimd.tensor_tensor(
                    out=gf[:, g4], in0=gw[:, g4], in1=pgm[:, g4], op=Alu.add
                )
                # dist^2 = hr + g - 2*dot, as (hr - dot) - dot + g so the
                # whole chain stays on Pool-legal add/sub ops
                nc.gpsimd.tensor_tensor(
                    out=s2a[:, g4], in0=hr_sb[:, g4], in1=dotf[:, g4],
                    op=Alu.subtract,
                )
                nc.gpsimd.tensor_tensor(
                    out=s2b[:, g4], in0=s2a[:, g4], in1=dotf[:, g4],
                    op=Alu.subtract,
                )
                nc.gpsimd.tensor_tensor(
                    out=dist2[:, g4], in0=s2b[:, g4], in1=gf[:, g4],
                    op=Alu.add,
                )

            def tail_rest_dve(g4, with_mult=True):
                # drain variant: the pack stream is over, the DVE is idle --
                # run the whole chain (incl. stage-1 mult) on it with zero
                # cross-engine hops (GpSimd is busy with the last gathers)
                if with_mult:
                    nc.vector.tensor_tensor(
                        out=prj[:, g4, :, :],
                        in0=vr2_sb[:, g4, :],
                        in1=gat[:, g4, 0 : 2 * D],
                        op=Alu.mult,
                    )
                nc.vector.tensor_reduce(
                    dq[:, g4, :], prj[:, g4, :, :],
                    axis=mybir.AxisListType.X, op=Alu.add,
                )
                nc.vector.tensor_tensor(
                    out=sel[:, g4], in0=dq[:, g4, 1], in1=dq[:, g4, 0],
                    op=Alu.is_gt,
                )
                nc.vector.tensor_tensor(
                    out=dotw[:, g4], in0=dq[:, g4, 0], in1=dq[:, g4, 1],
                    op=Alu.max,
                )
                nc.vector.tensor_tensor(
                    out=c2[:, g4], in0=pd_sb[:, g4], in1=dotw[:, g4],
                    op=Alu.is_gt,
                )
                nc.vector.tensor_tensor(
                    out=dotf[:, g4], in0=dotw[:, g4], in1=pd_sb[:, g4],
                    op=Alu.max,
                )
                nc.vector.tensor_tensor(
                    out=gdf[:, g4], in0=gat[:, g4, 2 * D + 1],
                    in1=gat[:, g4, 2 * D], op=Alu.subtract,
                )
                nc.vector.tensor_tensor(
                    out=gm[:, g4], in0=gdf[:, g4], in1=sel[:, g4], op=Alu.mult
                )
                nc.vector.tensor_tensor(
                    out=gw[:, g4], in0=gat[:, g4, 2 * D], in1=gm[:, g4],
                    op=Alu.add,
                )
                nc.vector.tensor_tensor(
                    out=pgd[:, g4], in0=pg_sb[:, g4], in1=gw[:, g4],
                    op=Alu.subtract,
                )
                nc.vector.tensor_tensor(
                    out=pgm[:, g4], in0=pgd[:, g4], in1=c2[:, g4], op=Alu.mult
                )
                nc.vector.tensor_tensor(
                    out=gf[:, g4], in0=gw[:, g4], in1=pgm[:, g4], op=Alu.add
                )
                nc.vector.scalar_tensor_tensor(
                    out=s2a[:, g4], in0=dotf[:, g4], scalar=-2.0,
                    in1=gf[:, g4], op0=Alu.mult, op1=Alu.add,
                )
                nc.vector.tensor_tensor(
                    out=dist2[:, g4], in0=s2a[:, g4], in1=hr_sb[:, g4],
                    op=Alu.add,
                )

            lns = small.tile([P, NT], f32)
            kol = small.tile([P, NT], f32)

            for s in range(NT // 2):
                tA, tB = 2 * s, 2 * s + 1
                for w in range(NW):
                    wave_pair(s, w)
                    # combines lag the pack stream: stage 1 for the 2 tiles
                    # finished at s-1, stage 2 one s-iteration later.  Both
                    # are emitted mid-iteration (after waves 1 and 3) so
                    # they sit in the GpSimd queue ahead of this iteration's
                    # own unpack/gather burst -- that keeps prj(12,13) ready
                    # before the drain and stage-2's DVE ops stall-free
                    if w == 1 and s >= 1:
                        tail_mult(slice(2 * s - 2, 2 * s))
                    if w == 3 and s >= 2:
                        tail_rest(slice(2 * s - 4, 2 * s - 2))

            # dist2 for tiles 0-11 is final before the drain: ship their
            # koleo values while the last gathers are still in flight
            nc.scalar.activation(lns[:, 0:12], dist2[:, 0:12], Act.Ln)
            nc.scalar.activation(
                kol[:, 0:12], lns[:, 0:12], Act.Relu,
                bias=mlnn_sb[:, 0:1], scale=-0.5,
            )
            nc.sync.dma_start(out_d[:, 0:12], kol[:, 0:12])

            # drain: tiles 12/13 combine while gathers 14/15 finish (their
            # stage-1 mult was emitted at s=7 w1); 14/15 run as ONE 2-wide
            # chain -- the DVE is in-order so two per-tile chains would
            # serialize anyway, and both gathers land within ~1us of each
            # other, so one chain over both tiles is ~1us shorter
            tail_rest_dve(slice(12, 14), with_mult=False)
            tail_rest_dve(slice(14, NT))
            nc.scalar.activation(lns[:, 12:NT], dist2[:, 12:NT], Act.Ln)
            # koleo = relu(-0.5*ln(dist^2) - ln(N))
            nc.scalar.activation(
                kol[:, 12:NT], lns[:, 12:NT], Act.Relu,
                bias=mlnn_sb[:, 0:1], scale=-0.5,
            )
            # gpsimd SWDGE is ~2us cheaper than SP's HWDGE for this last
            # latency-exposed 2KB store
            nc.gpsimd.dma_start(out_d[:, 12:NT], kol[:, 12:NT])
            if DEBUG:
                nc.sync.dma_start(dbg_bm[:], bm[:])
                nc.sync.dma_start(dbg_pf[:], pfi[:])
                nc.sync.dma_start(dbg_dq[:], dq[:])
                nc.sync.dma_start(dbg_d2[:], dist2[:])
                nc.sync.dma_start(dbg_g[:, :, 0], gat[:, :, 128])
                nc.sync.dma_start(dbg_g[:, :, 1], gat[:, :, 129])
                nc.sync.dma_start(dbg_g[:, :, 2], gat[:, :, 0])
                nc.sync.dma_start(dbg_g[:, :, 3], gat[:, :, D])


    nc.compile()
    _built["nc"] = nc
    return nc


def _prep_in_maps(v: np.ndarray) -> list[dict]:
    bf = ml_dtypes.bfloat16
    f8 = ml_dtypes.float8_e4m3
    in_maps = []
    jj = np.arange(ROWS)
    for c in range(NCORES):
        vr = np.roll(v, -c * ROWS, axis=0)
        w = (vr[0::2] + vr[1::2]) * np.float32(SCALE_W)  # [NPAIR, D]
        u = (vr[0::2] - vr[1::2]) * np.float32(SCALE_W)
        rows = vr[:ROWS]
        wt = np.ascontiguousarray(w.T).astype(f8)
        ut = np.ascontiguousarray(u.T).astype(f8)
        rt = np.ascontiguousarray(rows.T * np.float32(SCALE_R)).astype(bf)
        rsb = rows.reshape(NT, P, D).transpose(1, 0, 2)

        n2 = np.sum(vr.astype(np.float64) * vr, axis=1).astype(np.float32)
        Sv = np.sum(vr.astype(np.float64), axis=1).astype(np.float32)
        gvec = n2 - np.float32(2.0 * EPS) * Sv  # per-row: ||x||^2 - 2 eps sum(x)

        vpair = np.zeros((NPAIR, VREC), dtype=np.float32)
        vpair[:, 0:D] = vr[0::2]
        vpair[:, D : 2 * D] = vr[1::2]
        vpair[:, 2 * D] = gvec[0::2]
        vpair[:, 2 * D + 1] = gvec[1::2]

        hr = (
            n2[:ROWS]
            + np.float32(2.0 * EPS) * Sv[:ROWS]
            + np.float32(D * EPS * EPS)
        )
        pidx = jj ^ 1
        pdot = np.sum(
            rows.astype(np.float64) * vr[pidx], axis=1
        ).astype(np.float32)
        pgv = gvec[pidx]

        in_maps.append(
            {
                "wt": np.concatenate([wt, wt], axis=0),
                "ut": np.concatenate([ut, ut], axis=0),
                "vrows_t": np.concatenate([rt, rt], axis=0),
                "vr2": np.ascontiguousarray(np.concatenate([rsb, rsb], axis=2)),
                "vpair": vpair,
                "hr": np.ascontiguousarray(hr.reshape(NT, P).T),
                "pd": np.ascontiguousarray(pdot.reshape(NT, P).T),
                "pg": np.ascontiguousarray(pgv.reshape(NT, P).T),
            }
        )
    return in_maps


# test.py can flip these to profile the run
TRACE = False
DEBUG = False
DEBUG_NO_CLAMP = False
LAST_RESULT = {}


def kernel(latents: np.ndarray) -> np.ndarray:
    from concourse.bass_utils import run_bass_kernel_spmd

    v = np.asarray(latents, dtype=np.float32).reshape(N, D)
    nc = _build_nc()
    in_maps = _prep_in_maps(v)

    kwargs = {}
    if TRACE:
        kwargs = dict(trace=True, stitch_traces=False)
    res = run_bass_kernel_spmd(nc, in_maps, core_ids=list(range(NCORES)), **kwargs)
    LAST_RESULT["res"] = res

    vals = np.concatenate([r["out"].reshape(-1) for r in res.results])
    return np.array(np.mean(vals), dtype=np.float32)
